# revision 1
# baseline (speedup 1.0000x reference)
"""MoE routing kernel for 8 Trainium2 NeuronCores.

Problem: B=65536 tokens, shared Linear(512->256)+ReLU, then per-token expert
MLP Linear(256->100)+ReLU -> Linear(100->1), expert chosen by idx in [0,16).

Strategy (expert-parallel, host-side routing):
  - Host sorts tokens by expert. Experts 2c and 2c+1 go to core c, each in a
    fixed-capacity slot of C tokens (C = max expert count rounded up to 8),
    padded with token 0 (padding outputs are computed then discarded).
  - Host pre-transposes x to [512, TOK] bf16 per core so the contraction dim
    (IN_DIM) lands on SBUF partitions: all three GEMMs then chain on-chip
    with no transposes (layer-1 out [hid, tok] feeds fc1, which feeds fc2).
  - Device, per group of <=512 tokens: 8 accumulating layer-1 matmuls
    (512-dim contraction, 2 hid chunks) + bias/ReLU (Vector/Scalar), 2
    matmuls for expert FC1 + bias/ReLU, 1 matmul for FC2 (b2 folded via a
    ones row of h1; stationary zero-padded to 128 cols so the PE tile
    config never changes, avoiding ~190ns/group reconfig bubbles). Up to 3
    groups' FC2 results accumulate into one PSUM bank at partitions
    0/32/64, then one copy + one DMA out per block.
  - The PE stream is a 2-deep software pipeline: iteration i runs L1(i),
    fc1(i-1), fc2(i-2), so every ReLU has >=1.5us of slack before its
    consuming matmul and the in-order PE never stalls on Vector/Scalar.
  - Schedule: slot A's remainder first, then six 256-token half-groups
    (early DMA supply is only ~250GB/s until the backlog builds; small
    groups keep the PE fed), then full groups; slot B's remainder last so
    the tail chain is short. Warm-up matmuls (never read, uninitialized
    operands) keep the PE clock ramped from program start until the first
    x tile lands (an idle PE drops to half speed).
  - DMA: x + w1 + out-blocks ride the sync HWDGE queue in need-order; ws
    rides the scalar HWDGE queue; tiny biases + fc2 columns ride the
    gpsimd software DGE. ws/w1 are pre-packed partition-major on the host
    so their DMAs are contiguous 1-2KB rows (512B gather packets are ~4x
    slower). The fc2 stationary tiles (mostly zeros) are assembled on-chip
    from a 1.5KB DMA to save scarce early DMA-budget bytes.
  - Weights (tiny) live resident in SBUF in bf16; PSUM accumulates fp32.
"""

import math
import os
import sys

import numpy as np

for _p in ("/opt/trn_rl_repo", "/opt/pypackages"):
    if _p not in sys.path and os.path.isdir(_p):
        sys.path.append(_p)

import ml_dtypes

BF16 = ml_dtypes.bfloat16

B, IN_DIM, HID, EXP_HID, OUT_DIM, N_EXP = 65536, 512, 256, 100, 1, 16
N_CORES = 8
GROUP = 512  # tokens per matmul group (= PSUM bank free-dim in fp32)

_PROGRAM_CACHE = {}


def _block_schedule(CA: int, CB: int):
    """Execution-order blocks: (exec_idx, expert_slot, token_offset, ntok).

    Slots have asymmetric capacities (big expert paired with small expert
    per core, so CA+CB < 2*max_count). Slot A's short remainder first,
    then six 256-token half-groups (early DMA supply is only ~250GB/s
    until the backlog builds; small groups keep the PE fed), then full
    groups; slot B's remainder last."""
    n_full_a, r_a = CA // GROUP, CA % GROUP
    n_full_b, r_b = CB // GROUP, CB % GROUP
    blocks = []
    if r_a:
        blocks.append((0, r_a))
    n_half = min(3, n_full_a - 1)
    blocks += [(0, GROUP // 2)] * (2 * n_half)
    blocks += [(0, GROUP)] * (n_full_a - n_half)
    blocks += [(1, GROUP)] * n_full_b
    if r_b:
        blocks.append((1, r_b))
    # taper the tail: the final groups' fc2 matmuls sit at the end of the
    # PE stream with little covering work, so their h1-ReLU stall scales
    # with group size. End with a 128-token group (and keep the one before
    # it moderate) to shrink the tail chain.
    slot_last, n_last = blocks[-1]
    if n_last > 256:
        blocks[-1] = (slot_last, n_last - 128)
        blocks.append((slot_last, 128))
    out = []
    off = [0, 0]
    base = [0, CA]
    # offsets: each slot's remainder lives at the end of its range even
    # when executed first; assign offsets in per-slot execution order.
    for slot, n in blocks:
        out.append((len(out), slot, base[slot] + off[slot], n))
        off[slot] += n
    return out


def _fc2_blocks(n_groups: int):
    """fc2 PSUM-sharing blocks: chunks of 3 exec-groups, but keep the last
    TWO blocks singletons so the final copies run on two engines in
    parallel and the final DMAs are tiny (short tail)."""
    blocks = []
    i = 0
    while i < n_groups - 2:
        take = min(3, n_groups - 2 - i)
        blocks.append(list(range(i, i + take)))
        i += take
    blocks.append([n_groups - 2])
    blocks.append([n_groups - 1])
    return blocks


def _build_program(CA: int, CB: int):
    """Build (and cache) the Bass program for slot capacities (CA, CB)."""
    import concourse.bass as bass
    import concourse.mybir as mybir
    import concourse.tile as tile
    from concourse import bacc

    f32 = mybir.dt.float32
    bf16 = mybir.dt.bfloat16
    AF = mybir.ActivationFunctionType
    ALU = mybir.AluOpType

    nc = bacc.Bacc("TRN2", target_bir_lowering=False, debug=False)

    groups = _block_schedule(CA, CB)
    n_groups = len(groups)
    fc2_blocks = _fc2_blocks(n_groups)
    n_blk = len(fc2_blocks)
    blk_of = {}
    for b, js in enumerate(fc2_blocks):
        for lane, j in enumerate(js):
            blk_of[j] = (b, lane, lane == len(js) - 1)

    # x pre-blocked on host in EXECUTION order:
    # xg[g, p, kc*512+t] = x[token of exec-group g at pos t, kc*128+p]
    xg_d = nc.dram_tensor(
        "xg", [n_groups, 128, 4 * GROUP], bf16, kind="ExternalInput"
    ).ap()
    # ws pre-packed p-major on host: ws[p, kc*HID + m] = Ws[kc*128+p, m], so
    # the DMA is fully contiguous (2KB rows) instead of 512B gather packets
    ws_d = nc.dram_tensor("ws", [128, 4 * HID], bf16, kind="ExternalInput").ap()
    bs_d = nc.dram_tensor("bs", [128, 2], f32, kind="ExternalInput").ap()
    # w1 padded to 128 output cols (100 real) so FWL kicks in on LDWEIGHTS
    w1_d = nc.dram_tensor("w1", [128, 2 * 2 * 128], bf16, kind="ExternalInput").ap()
    # only the real fc2 columns come from DRAM (1.5KB); the mostly-zero
    # 128-wide stationary tiles are assembled on-chip (saves 190KB of
    # scarce early DMA budget)
    w2c_d = nc.dram_tensor("w2c", [128, 2 * 3], bf16, kind="ExternalInput").ap()
    # b1 rows 0..99 = b1[e]; rows 100..127 = 1.0 so relu(0 + 1) makes a ones
    # row block that w2's bias row consumes (fc2 bias folded into the matmul)
    b1_d = nc.dram_tensor("b1", [128, 2], f32, kind="ExternalInput").ap()
    # w2L[p, e, lane, m]: col m = 32*lane holds W2[e,:,0] rows (+ b2[e] at
    # row 100), all other cols zero. A 128-wide stationary operand keeps the
    # PE tile config at 128x128 for every matmul in the program (a 128x32
    # fc2 tile would force a pipeline-draining tile reconfig twice per
    # group, ~190ns); the zero columns let 3 groups accumulate into one
    # PSUM bank at partitions 0/32/64 without clobbering each other.
    # out[blk, lane, t]: exec-group g = blk*4+lane, token t of that group
    out_d = nc.dram_tensor("out", [n_blk, 3, GROUP], f32, kind="ExternalOutput").ap()

    with tile.TileContext(nc) as tc:
        with (
            tc.tile_pool(name="const", bufs=1) as const,
            tc.tile_pool(name="xp", bufs=24) as xp,
            tc.tile_pool(name="hp", bufs=5) as hp,
            tc.tile_pool(name="h1p", bufs=5) as h1p,
            tc.tile_pool(name="ob", bufs=3) as obp,
            tc.tile_pool(name="ps1", bufs=4, space="PSUM") as ps1,
            tc.tile_pool(name="ps2", bufs=2, space="PSUM") as ps2,
            tc.tile_pool(name="ps3", bufs=2, space="PSUM") as ps3,
        ):
            ws_sb = const.tile([128, 4, HID], bf16)
            bs_sb = const.tile([128, 2], f32)
            w1_sb = const.tile([128, 2, 2, 128], bf16)
            b1_sb = const.tile([128, 2], f32)
            w2_sb = const.tile([128, 2, 3, 128], bf16)
            w2c_sb = const.tile([128, 2, 3], bf16)
            warm_w = const.tile([128, GROUP], bf16)
            x_tiles = []

            # All x tiles up front (allocation is metadata-only; the DMA
            # start is what gets scheduled). All x groups ride the sync
            # HWDGE queue, unsplit (one ~650ns post per group keeps the
            # supply rate at ~330GB/s).
            for i, (bi, _, _, _) in enumerate(groups):
                x_tiles.append(
                    xp.tile([128, 4, GROUP], bf16, tag="x", name=f"x_sb{bi}")
                )

            def post_x(i, eng, half=None):
                bi, _, _, n = groups[i]
                x_sb = x_tiles[i]
                xg_v = xg_d[bi].rearrange("p (c t) -> p c t", c=4)
                if half is not None:
                    ks = slice(0, 2) if half == 0 else slice(2, 4)
                    eng.dma_start(x_sb[:, ks, :n], xg_v[:, ks, :n])
                elif n == GROUP:
                    eng.dma_start(x_sb.rearrange("p c t -> p (c t)"), xg_d[bi])
                else:
                    eng.dma_start(x_sb[:, :, :n], xg_v[:, :, :n])

            # Early DMA budget is ~250GB/s shared across ALL queues, so
            # order transfers by need-time and ship the minimum: ws hc0
            # half + first x groups first; ws hc1 and w1 slot in where
            # needed; tiny biases + fc2 columns ride the gpsimd SWDGE.
            nc.scalar.dma_start(ws_sb.rearrange("p c m -> p (c m)"), ws_d)
            post_x(0, nc.sync)
            post_x(1, nc.sync)
            # only slot A's w1 half (64KB) rides the scarce early byte
            # budget; slot B's half is not needed until ~35us and drips in
            # over the gpsimd software DGE
            w1_dv = w1_d.rearrange("p (e x) -> p e x", e=2)
            nc.sync.dma_start(w1_sb[:, 0:1, :, :], w1_dv[:, 0:1, :])
            post_x(2, nc.sync)
            post_x(3, nc.sync)
            post_x(4, nc.sync)
            nc.gpsimd.dma_start(bs_sb[:, :], bs_d[:, :])
            nc.gpsimd.dma_start(b1_sb[:, :], b1_d[:, :])
            nc.gpsimd.dma_start(w2c_sb[:, :, :], w2c_d)
            nc.gpsimd.dma_start(w1_sb[:, 1:2, :, :], w1_dv[:, 1:2, :])
            for i in range(5, len(groups)):
                post_x(i, nc.sync)

            # PE warm-up: full-width matmuls that keep the PE continuously
            # busy from program start until the first x tile lands, so the
            # PE clock (p-state) is fully ramped when real work begins. An
            # idle PE drops back to half speed (~427ns/512-col matmul).
            # Results are never read; warm_w is deliberately uninitialized
            # (garbage values are harmless: the PSUM result is never read,
            # and skipping the memset removes the vector-engine dependency
            # from the PE's start, ~0.9us earlier first matmul).
            warm_p = ps1.tile([128, GROUP], f32, tag="p1", name="warm_p")
            for _ in range(7):
                nc.tensor.matmul(
                    warm_p[:, :], warm_w[:, :128], warm_w[:, :], start=True, stop=True
                )

            # assemble the zero-padded 128-wide fc2 stationary tiles on-chip
            nc.vector.memset(w2_sb.rearrange("p e l m -> p (e l m)"), 0.0)
            for e_ in range(2):
                for lane_ in range(3):
                    nc.scalar.copy(
                        w2_sb[:, e_, lane_, 32 * lane_ : 32 * lane_ + 1],
                        w2c_sb[:, e_, lane_ : lane_ + 1],
                    )

            # Two-deep software pipeline over the PE stream: iteration i runs
            # L1(i), fc1(i-1), fc2(i-2). Every activation (h, h1) then has
            # >=1.5us between its producing ReLU and its consuming matmul, so
            # the in-order PE stream never stalls on the Vector/Scalar
            # engines.
            h_tiles = {}
            h1_tiles = {}
            p3 = None

            def do_l1(i):
                _, _, _, n = groups[i]
                x_sb = x_tiles[i]
                h_sb = hp.tile([128, 2, GROUP], bf16, tag="h")
                for hc in range(2):
                    p1 = ps1.tile([128, GROUP], f32, tag="p1")
                    for kc in range(4):
                        nc.tensor.matmul(
                            p1[:, :n],
                            ws_sb[:, kc, hc * 128 : (hc + 1) * 128],
                            x_sb[:, kc, :n],
                            start=(kc == 0),
                            stop=(kc == 3),
                        )
                    # h = relu(psum + bs): hc0 on VectorE, hc1 on ScalarE
                    if hc == 0:
                        nc.vector.tensor_scalar(
                            h_sb[:, hc, :n],
                            p1[:, :n],
                            bs_sb[:, hc : hc + 1],
                            0.0,
                            ALU.add,
                            ALU.max,
                        )
                    else:
                        nc.scalar.activation(
                            h_sb[:, hc, :n],
                            p1[:, :n],
                            AF.Relu,
                            bias=bs_sb[:, hc : hc + 1],
                        )
                h_tiles[i] = h_sb

            def do_fc1(j):
                if j < 0 or j >= len(groups):
                    return
                _, e, _, n = groups[j]
                h_sb = h_tiles.pop(j)
                p2 = ps2.tile([128, GROUP], f32, tag="p2")
                for kc in range(2):
                    nc.tensor.matmul(
                        p2[:, :n],
                        w1_sb[:, e, kc, :],
                        h_sb[:, kc, :n],
                        start=(kc == 0),
                        stop=(kc == 1),
                    )
                # h1 rows 0..99 = relu(psum + b1); rows 100..127 = relu(0+1)=1
                # alternate engines so neither Vector nor Scalar is loaded 2x
                h1_sb = h1p.tile([128, GROUP], bf16, tag="h1")
                if j % 2 == 0:
                    nc.vector.tensor_scalar(
                        h1_sb[:, :n],
                        p2[:, :n],
                        b1_sb[:, e : e + 1],
                        0.0,
                        ALU.add,
                        ALU.max,
                    )
                else:
                    nc.scalar.activation(
                        h1_sb[:, :n],
                        p2[:, :n],
                        AF.Relu,
                        bias=b1_sb[:, e : e + 1],
                    )
                h1_tiles[j] = h1_sb

            def issue_fc2(j):
                nonlocal p3
                if j < 0 or j >= len(groups):
                    return
                _, e, _, n = groups[j]
                # up to 3 exec-groups share one PSUM bank at partitions
                # 0/32/64 (PE col-tile base must be 0/32/64)
                blk, lane, last = blk_of[j]
                if lane == 0:
                    p3 = ps3.tile([128, GROUP], f32, tag="p3")
                nc.tensor.matmul(
                    p3[:, :n],
                    w2_sb[:, e, lane, :],
                    h1_tiles.pop(j)[:, :n],
                    start=(lane == 0),
                    stop=last,
                )
                if last:
                    ob = obp.tile([128, GROUP], f32, tag="ob")
                    nrows = 32 * lane + 1
                    ncols = GROUP if lane > 0 else n
                    if blk == len(fc2_blocks) - 2:
                        # run the two final copies on different engines so
                        # they overlap (shorter tail chain)
                        nc.vector.tensor_scalar(
                            ob[:nrows, :ncols],
                            p3[:nrows, :ncols],
                            0.0,
                            0.0,
                            ALU.add,
                            ALU.bypass,
                        )
                    else:
                        nc.scalar.copy(ob[:nrows, :ncols], p3[:nrows, :ncols])
                    # rows 0/32/64 of ob hold the lanes' fc2 outputs
                    ob_lanes = ob.rearrange("(l s) t -> l s t", s=32)[:, 0, :]
                    nc.sync.dma_start(
                        out_d[blk, : lane + 1, :ncols], ob_lanes[: lane + 1, :ncols]
                    )

            for i in range(len(groups)):
                do_l1(i)
                do_fc1(i - 1)
                issue_fc2(i - 2)
            do_fc1(len(groups) - 1)
            issue_fc2(len(groups) - 2)
            # keep the sync DMA queue streaming so the final (tiny) out
            # block doesn't pay the ~1.6us queue-restart doorbell latency
            nc.sync.dma_start(warm_w.rearrange("p t -> p t"), xg_d[0][:, : GROUP])
            issue_fc2(len(groups) - 1)

    nc.compile()
    return nc


def _get_program(CA: int, CB: int):
    if (CA, CB) not in _PROGRAM_CACHE:
        _PROGRAM_CACHE[(CA, CB)] = _build_program(CA, CB)
    return _PROGRAM_CACHE[(CA, CB)]


def kernel(x, idx, Ws, bs, W1, b1, W2, b2, _trace=False, _result_box=None):
    from concourse.bass_utils import run_bass_kernel_spmd

    x = np.asarray(x)
    idx = np.asarray(idx).astype(np.int64)
    Ws = np.asarray(Ws, dtype=np.float32)
    bs = np.asarray(bs, dtype=np.float32)
    W1 = np.asarray(W1, dtype=np.float32)
    b1 = np.asarray(b1, dtype=np.float32)
    W2 = np.asarray(W2, dtype=np.float32)
    b2 = np.asarray(b2, dtype=np.float32)

    counts = np.bincount(idx, minlength=N_EXP)
    # pair the biggest expert with the smallest per core: slot capacities
    # then shrink to the max of each half instead of the global max
    eorder = np.argsort(-counts, kind="stable")
    big, small = eorder[:N_CORES], eorder[: N_CORES - 1 : -1]
    CA = max(2 * GROUP, int(math.ceil(counts[big].max() / 8) * 8))
    CB = max(2 * GROUP, int(math.ceil(counts[small].max() / 8) * 8))
    nc = _get_program(CA, CB)
    groups = _block_schedule(CA, CB)
    n_groups = len(groups)
    fc2_blocks = _fc2_blocks(n_groups)

    order = np.argsort(idx, kind="stable")
    bounds = np.zeros(N_EXP + 1, dtype=np.int64)
    np.cumsum(counts, out=bounds[1:])
    tok_by_expert = [order[bounds[e] : bounds[e + 1]] for e in range(N_EXP)]

    # shared-layer weights, chunked for the device (same for every core)
    # p-major: ws_host[p, kc, m] = Ws[kc*128+p, m]
    ws_host = np.ascontiguousarray(
        Ws.reshape(4, 128, HID).transpose(1, 0, 2).reshape(128, 4 * HID)
    ).astype(BF16)
    bs_host = np.ascontiguousarray(bs.reshape(2, 128).T).astype(np.float32)

    x_bf = x.astype(BF16)
    in_maps = []
    core_tokens = []
    for c in range(N_CORES):
        ea, eb = int(big[c]), int(small[c])
        # per-slot token lists padded to capacity with token 0
        toks = np.zeros(CA + CB, dtype=np.int64)
        toks[: counts[ea]] = tok_by_expert[ea]
        toks[CA : CA + counts[eb]] = tok_by_expert[eb]
        core_tokens.append(toks)

        # execution-order groups: gather each group's tokens
        toks_p = np.zeros(n_groups * GROUP, dtype=np.int64)
        for g, (_, slot, off, n) in enumerate(groups):
            toks_p[g * GROUP : g * GROUP + n] = toks[off : off + n]
        xg = np.ascontiguousarray(
            x_bf[toks_p].reshape(n_groups, GROUP, 4, 128).transpose(0, 3, 2, 1)
        ).reshape(n_groups, 128, 4 * GROUP)

        w1_pair = np.zeros((2, 2, 128, 128), dtype=BF16)
        w1_pair[:, :, :, :EXP_HID] = W1[[ea, eb]].reshape(2, 2, 128, EXP_HID).astype(BF16)
        # p-major contiguous: w1_pair[p, e, kc, m]
        w1_pair = np.ascontiguousarray(w1_pair.transpose(2, 0, 1, 3)).reshape(
            128, 2 * 2 * 128
        )
        b1_pair = np.ones((128, 2), dtype=np.float32)
        b1_pair[:EXP_HID] = b1[[ea, eb]].T
        w2_pair = np.zeros((128, 2, 3), dtype=BF16)
        for s_ in range(2):
            col = np.zeros(128, dtype=np.float32)
            col[:EXP_HID] = W2[[ea, eb][s_], :, 0]
            col[EXP_HID] = b2[[ea, eb][s_], 0]
            for lane in range(3):
                w2_pair[:, s_, lane] = col.astype(BF16)
        w2_pair = w2_pair.reshape(128, 2 * 3)

        in_maps.append(
            {
                "xg": xg,
                "ws": ws_host,
                "bs": bs_host,
                "w1": w1_pair,
                "b1": b1_pair,
                "w2c": w2_pair,
            }
        )

    res = run_bass_kernel_spmd(
        nc,
        in_maps,
        core_ids=list(range(N_CORES)),
        trace=_trace,
        **({"trace_cores": [0]} if _trace else {}),
    )
    if _result_box is not None:
        _result_box.append(res)

    out = np.zeros((B, OUT_DIM), dtype=np.float32)
    for c in range(N_CORES):
        ea, eb = int(big[c]), int(small[c])
        oc = res.results[c]["out"]  # [n_blk, 3, GROUP]
        # scatter back: group g's cols [0, n) are slot tokens [off, off+n)
        vals = np.zeros(CA + CB, dtype=np.float32)
        for b, js in enumerate(fc2_blocks):
            for lane, g in enumerate(js):
                _, slot, off, n = groups[g]
                vals[off : off + n] = oc[b, lane, :n]
        out[core_tokens[c][: counts[ea]], 0] = vals[: counts[ea]]
        out[core_tokens[c][CA : CA + counts[eb]], 0] = vals[CA : CA + counts[eb]]
    return out

